# revision 5
# baseline (speedup 1.0000x reference)
"""Trainium2 Bass kernel for CrossModalAttention.

Reference computation (per (b, m) of B=4 x M=3):
    Q = x_q @ Wq.T + bq ; K = x_k @ Wk.T + bk ; V = x_v @ Wv.T + bv
    per head h (4 heads of dim 128):
        scores = Q_h @ K_h.T / sqrt(128)      [2048, 2048]
        attn   = softmax(scores, axis=-1)
        out_h  = attn @ V_h                   [2048, 128]

Sharding over 8 cores: 48 (b*m, head) units, 6 per core.
  core c: slot A = bm c      (all 4 heads)
          slot B = bm 8+c//2 (heads {0,1} if c even else {2,3})

On-device layout strategy per slot:
  - inputs are loaded transposed (xT: [c,512] on partitions) via bf16
    xbar DMA-transpose directly from HBM
  - QT, KT computed as [d, tok] (head dim on partitions), V as [tok, d]
  - scores are computed TRANSPOSED (ST[k, q] = K @ Q.T) so that the
    attn @ V contraction over k can use V tiles as the stationary
    operand with no transposes of the [2048, 2048] attention matrix
  - softmax denominator: free-axis tree-sum over k-tiles on DVE +
    one ones-matmul for the partition-axis sum (result is broadcast
    over all partitions by construction)
  - no max-subtraction: scores are O(1) here, exp cannot overflow,
    and softmax is shift-invariant
  - final out.T [d, q] chunks are transposed back via PE transpose
"""

import sys
import os

for _p in ("/root/.axon_site/_ro/trn_rl_repo", "/opt/trn_rl_repo"):
    if os.path.isdir(_p) and _p not in sys.path:
        sys.path.append(_p)

import numpy as np
import ml_dtypes

import concourse.bass as bass
import concourse.tile as tile
from concourse import bacc, mybir
from concourse.bass_utils import run_bass_kernel_spmd
from concourse.masks import make_identity

B, M, NTOK, DIM = 4, 3, 2048, 512
H, HD = 4, 128
NBM = B * M  # 12
NCORES = 8
SCALE = 1.0 / float(np.sqrt(HD))

F32 = mybir.dt.float32
F32R = mybir.dt.float32r
BF16 = mybir.dt.bfloat16

TT = NTOK // 128  # 16 token tiles
CT = DIM // 128  # 4 contraction tiles
QCH = 512  # q is processed in chunks of 512
NQC = NTOK // QCH  # 4

# Knobs the test harness may flip before calling kernel():
TRACE = False
TRACE_KWARGS = {}
LAST_RESULTS = None


def _emit_slot(nc, pools, dram, s, nh, ident, ones):
    """Emit instructions for one (bm, head-set) slot. nh = number of heads."""
    D = nh * HD
    (xtp, qkvp, wp, ep, accp, recp, outp, biasp, pst, ppv, pden, ptp) = pools
    out_d = dram[f"out_{s}"]

    # ---- biases ----
    # bq/bk laid out [p, which, head] so [*, i, dt:dt+1] is a per-partition
    # scalar for head dt; bv broadcast along partitions (added along free).
    bqk = biasp.tile([128, 2, nh], F32, tag="bqk")
    nc.sync.dma_start(
        out=bqk[:, 0, :], in_=dram[f"bq_{s}"][:].rearrange("(j p) -> p j", p=128)
    )
    nc.sync.dma_start(
        out=bqk[:, 1, :], in_=dram[f"bk_{s}"][:].rearrange("(j p) -> p j", p=128)
    )
    bvb = biasp.tile([128, D], F32, tag="bvb")
    nc.sync.dma_start(out=bvb[:, :], in_=dram[f"bv_{s}"][:].unsqueeze(0).to_broadcast([128, D]))

    # ---- projections ----
    QT = qkvp.tile([128, nh, NTOK], BF16, tag="qt")  # [d, head, tok]
    KT = qkvp.tile([128, nh, NTOK], BF16, tag="kt")
    V = qkvp.tile([128, TT, D], BF16, tag="v")  # [tok, ttile, d]

    for which, (xname, wname, dst) in enumerate(
        (("xq", "wq", QT), ("xk", "wk", KT))
    ):
        xt = xtp.tile([128, CT, NTOK], BF16, tag="xt")  # transposed input
        xr = dram[f"{xname}_{s}"][:].rearrange("M (c p) -> M c p", p=128)
        for ct in range(CT):
            nc.sync.dma_start(out=xt[:, ct], in_=xr[:, ct], transpose=True)
        w = wp.tile([128, CT, D], BF16, tag=wname)
        nc.sync.dma_start(
            out=w[:, :, :],
            in_=dram[f"{wname}_{s}"][:].rearrange("(c p) d -> p c d", p=128),
        )
        # dst[d, tok] = sum_c w[c, d] * xt[c, tok]  (+ bias[d])
        for dt in range(nh):
            for qc in range(NQC):
                ps = ppv.tile([128, QCH], F32, tag="pv")
                for ct in range(CT):
                    nc.tensor.matmul(
                        ps[:, :],
                        w[:, ct, dt * 128 : (dt + 1) * 128],
                        xt[:, ct, qc * QCH : (qc + 1) * QCH],
                        start=(ct == 0),
                        stop=(ct == CT - 1),
                    )
                nc.vector.tensor_scalar_add(
                    dst[:, dt, qc * QCH : (qc + 1) * QCH],
                    ps[:, :],
                    bqk[:, which, dt : dt + 1],
                )

    # V natural layout: V[tok, d] = sum_c xt[c, tok] * w[c, d] (+ bv[d])
    xt = xtp.tile([128, CT, NTOK], BF16, tag="xt")
    xr = dram[f"xv_{s}"][:].rearrange("M (c p) -> M c p", p=128)
    for ct in range(CT):
        nc.sync.dma_start(out=xt[:, ct], in_=xr[:, ct], transpose=True)
    w = wp.tile([128, CT, D], BF16, tag="wv")
    nc.sync.dma_start(
        out=w[:, :, :], in_=dram[f"wv_{s}"][:].rearrange("(c p) d -> p c d", p=128)
    )
    for tt in range(TT):
        ps = ppv.tile([128, D], F32, tag="pv")
        for ct in range(CT):
            nc.tensor.matmul(
                ps[:, :],
                xt[:, ct, tt * 128 : (tt + 1) * 128],
                w[:, ct, :],
                start=(ct == 0),
                stop=(ct == CT - 1),
            )
        nc.vector.tensor_add(V[:, tt, :], ps[:, :], bvb[:, :])

    # ---- attention ----
    for h in range(nh):
        for qc in range(NQC):
            qsl = slice(qc * QCH, (qc + 1) * QCH)
            # E[k, q] = exp(scale * sum_d KT[d, k] QT[d, q]), k-tiled
            E = ep.tile([128, TT, QCH], BF16, tag="E")
            for g in range(TT // 2):
                st = pst.tile([128, 2 * QCH], F32, tag="st")
                for j in range(2):
                    kt = 2 * g + j
                    nc.tensor.matmul(
                        st[:, j * QCH : (j + 1) * QCH],
                        KT[:, h, kt * 128 : (kt + 1) * 128],
                        QT[:, h, qsl],
                        start=True,
                        stop=True,
                    )
                nc.scalar.activation(
                    E[:, 2 * g : 2 * g + 2, :],
                    st[:, :].rearrange("p (a b) -> p a b", b=QCH),
                    mybir.ActivationFunctionType.Exp,
                    scale=SCALE,
                )
            # denominator: tree-sum over the 16 k-tiles (free axis) ...
            acc = accp.tile([128, 8, QCH], F32, tag="acc")
            nc.vector.tensor_add(acc[:, 0:8, :], E[:, 0:8, :], E[:, 8:16, :])
            nc.vector.tensor_add(acc[:, 0:4, :], acc[:, 0:4, :], acc[:, 4:8, :])
            nc.vector.tensor_add(acc[:, 0:2, :], acc[:, 0:2, :], acc[:, 2:4, :])
            nc.vector.tensor_add(acc[:, 0:1, :], acc[:, 0:1, :], acc[:, 1:2, :])
            # ... then partition-axis sum via ones-matmul; every output
            # partition receives the same row-sum (broadcast for free).
            den = pden.tile([128, QCH], F32, tag="den")
            nc.tensor.matmul(
                den[:, :], ones[:, :], acc[:, 0, :], start=True, stop=True
            )
            rec = recp.tile([128, QCH], F32, tag="rec")
            nc.vector.reciprocal(rec[:, :], den[:, :])

            # outT[d, q] = sum_k V[k, d] E[k, q]
            pv = ppv.tile([128, QCH], F32, tag="pv")
            for kt in range(TT):
                nc.tensor.matmul(
                    pv[:, :],
                    V[:, kt, h * 128 : (h + 1) * 128],
                    E[:, kt, :],
                    start=(kt == 0),
                    stop=(kt == TT - 1),
                )
            outT = recp.tile([128, QCH], F32, tag="outT")
            nc.vector.tensor_mul(outT[:, :], pv[:, :], rec[:, :])

            # transpose back to [q, d] and store
            ot = outp.tile([128, NQC, 128], F32, tag="ot")
            for j in range(NQC):
                tp = ptp.tile([128, 128], F32, tag="tp")
                nc.tensor.transpose(tp[:, :], outT[:, j * 128 : (j + 1) * 128], ident[:, :])
                nc.vector.tensor_copy(ot[:, j, :], tp[:, :])
            nc.sync.dma_start(
                out=out_d[qc * QCH : (qc + 1) * QCH, h * 128 : (h + 1) * 128].rearrange(
                    "(j p) d -> p j d", p=128
                ),
                in_=ot[:, :, :],
            )


def _build_program():
    # Bacc (not plain Bass): its compile() pipeline legalizes multi-wait
    # instructions (walrus accepts at most 1 sync wait per instruction).
    nc = bacc.Bacc()
    dram = {}
    for s in ("a", "b"):
        D = 512 if s == "a" else 256
        for nm in ("xq", "xk", "xv"):
            dram[f"{nm}_{s}"] = nc.dram_tensor(
                f"{nm}_{s}", [NTOK, DIM], BF16, kind="ExternalInput"
            )
        for nm in ("wq", "wk", "wv"):
            dram[f"{nm}_{s}"] = nc.dram_tensor(
                f"{nm}_{s}", [DIM, D], BF16, kind="ExternalInput"
            )
        for nm in ("bq", "bk", "bv"):
            dram[f"{nm}_{s}"] = nc.dram_tensor(
                f"{nm}_{s}", [D], F32, kind="ExternalInput"
            )
        dram[f"out_{s}"] = nc.dram_tensor(
            f"out_{s}", [NTOK, D], F32, kind="ExternalOutput"
        )

    with tile.TileContext(nc) as tc:
        with (
            tc.tile_pool(name="singles", bufs=1) as singles,
            tc.tile_pool(name="xtp", bufs=2) as xtp,
            tc.tile_pool(name="qkvp", bufs=1) as qkvp,
            tc.tile_pool(name="wp", bufs=1) as wp,
            tc.tile_pool(name="ep", bufs=2) as ep,
            tc.tile_pool(name="accp", bufs=2) as accp,
            tc.tile_pool(name="recp", bufs=2) as recp,
            tc.tile_pool(name="outp", bufs=3) as outp,
            tc.tile_pool(name="biasp", bufs=1) as biasp,
            tc.tile_pool(name="pst", bufs=2, space="PSUM") as pst,
            tc.tile_pool(name="ppv", bufs=2, space="PSUM") as ppv,
            tc.tile_pool(name="pden", bufs=1, space="PSUM") as pden,
            tc.tile_pool(name="ptp", bufs=1, space="PSUM") as ptp,
        ):
            ident = singles.tile([128, 128], F32, tag="ident")
            make_identity(nc, ident[:, :])
            ones = singles.tile([128, 128], F32, tag="ones")
            nc.vector.memset(ones[:, :], 1.0)

            pools = (xtp, qkvp, wp, ep, accp, recp, outp, biasp, pst, ppv, pden, ptp)
            _emit_slot(nc, pools, dram, "a", 4, ident, ones)
            _emit_slot(nc, pools, dram, "b", 2, ident, ones)

    # Run Bacc's compile pipeline (register allocation, sync-wait
    # legalization, nop fusion) — run_bass_via_pjrt does not call it.
    nc.finalize()
    return nc


_PROGRAM = None


def _get_program():
    global _PROGRAM
    if _PROGRAM is None:
        _PROGRAM = _build_program()
    return _PROGRAM


def kernel(query, key, value, Wq, bq, Wk, bk, Wv, bv):
    global LAST_RESULTS
    bf = ml_dtypes.bfloat16
    q = np.ascontiguousarray(np.asarray(query, np.float32).reshape(NBM, NTOK, DIM)).astype(bf)
    k = np.ascontiguousarray(np.asarray(key, np.float32).reshape(NBM, NTOK, DIM)).astype(bf)
    v = np.ascontiguousarray(np.asarray(value, np.float32).reshape(NBM, NTOK, DIM)).astype(bf)
    WqT = np.ascontiguousarray(np.asarray(Wq, np.float32).T).astype(bf)
    WkT = np.ascontiguousarray(np.asarray(Wk, np.float32).T).astype(bf)
    WvT = np.ascontiguousarray(np.asarray(Wv, np.float32).T).astype(bf)
    bq = np.asarray(bq, np.float32)
    bk = np.asarray(bk, np.float32)
    bv = np.asarray(bv, np.float32)

    in_maps = []
    for c in range(NCORES):
        bm_a = c
        bm_b = 8 + c // 2
        hs = (c % 2) * 256  # head-pair column offset for slot B
        in_maps.append(
            {
                "xq_a": q[bm_a], "xk_a": k[bm_a], "xv_a": v[bm_a],
                "xq_b": q[bm_b], "xk_b": k[bm_b], "xv_b": v[bm_b],
                "wq_a": WqT, "wk_a": WkT, "wv_a": WvT,
                "bq_a": bq, "bk_a": bk, "bv_a": bv,
                "wq_b": np.ascontiguousarray(WqT[:, hs : hs + 256]),
                "wk_b": np.ascontiguousarray(WkT[:, hs : hs + 256]),
                "wv_b": np.ascontiguousarray(WvT[:, hs : hs + 256]),
                "bq_b": np.ascontiguousarray(bq[hs : hs + 256]),
                "bk_b": np.ascontiguousarray(bk[hs : hs + 256]),
                "bv_b": np.ascontiguousarray(bv[hs : hs + 256]),
            }
        )

    nc = _get_program()
    res = run_bass_kernel_spmd(
        nc, in_maps, list(range(NCORES)), trace=TRACE, **TRACE_KWARGS
    )
    LAST_RESULTS = res

    out = np.empty((NBM, NTOK, DIM), np.float32)
    for c in range(NCORES):
        hs = (c % 2) * 256
        out[c] = res.results[c]["out_a"]
        out[8 + c // 2][:, hs : hs + 256] = res.results[c]["out_b"]
    return out.reshape(B, M, NTOK, DIM)


# revision 10
# speedup vs baseline: 1.2821x; 1.2821x over previous
"""Trainium2 Bass kernel for CrossModalAttention.

Reference computation (per (b, m) of B=4 x M=3):
    Q = x_q @ Wq.T + bq ; K = x_k @ Wk.T + bk ; V = x_v @ Wv.T + bv
    per head h (4 heads of dim 128):
        scores = Q_h @ K_h.T / sqrt(128)      [2048, 2048]
        attn   = softmax(scores, axis=-1)
        out_h  = attn @ V_h                   [2048, 128]

Sharding over 8 cores: 48 (b*m, head) units, 6 per core.
  core c: slot A = bm c      (all 4 heads)
          slot B = bm 8+c//2 (heads {0,1} if c even else {2,3})

On-device layout strategy per slot:
  - inputs are loaded transposed (xT: [c,512] on partitions) via bf16
    xbar DMA-transpose directly from HBM
  - QT, KT computed as [d, tok] (head dim on partitions), V as [tok, d]
  - scores are computed TRANSPOSED (ST[k, q] = K @ Q.T) so that the
    attn @ V contraction over k can use V tiles as the stationary
    operand with no transposes of the [2048, 2048] attention matrix
  - softmax denominator: free-axis tree-sum over k-tiles on DVE +
    one ones-matmul for the partition-axis sum (result is broadcast
    over all partitions by construction)
  - no max-subtraction: scores are O(1) here, exp cannot overflow,
    and softmax is shift-invariant
  - final out.T [d, q] chunks are transposed back via PE transpose
"""

import sys
import os

for _p in ("/root/.axon_site/_ro/trn_rl_repo", "/opt/trn_rl_repo"):
    if os.path.isdir(_p) and _p not in sys.path:
        sys.path.append(_p)

import numpy as np
import ml_dtypes

import concourse.bass as bass
import concourse.tile as tile
from concourse import bacc, mybir
from concourse.bass_utils import run_bass_kernel_spmd
from concourse.masks import make_identity

B, M, NTOK, DIM = 4, 3, 2048, 512
H, HD = 4, 128
NBM = B * M  # 12
NCORES = 8
SCALE = 1.0 / float(np.sqrt(HD))

F32 = mybir.dt.float32
F32R = mybir.dt.float32r
BF16 = mybir.dt.bfloat16

TT = NTOK // 128  # 16 token tiles
CT = DIM // 128  # 4 contraction tiles
QCH = 512  # q is processed in chunks of 512
NQC = NTOK // QCH  # 4

# Knobs the test harness may flip before calling kernel():
TRACE = False
TRACE_KWARGS = {}
LAST_RESULTS = None


def _emit_slot(nc, pools, dram, s, nh, ident, identb):
    """Emit instructions for one (bm, head-set) slot. nh = number of heads."""
    D = nh * HD
    (xtp, qkvp, wp, ep, accp, recp, outp, biasp, pst, ppv, ptp) = pools
    out_d = dram[f"out_{s}"]

    # ---- biases ----
    # bq/bk laid out [p, which, head] so [*, i, dt:dt+1] is a per-partition
    # scalar for head dt; bv broadcast along partitions (added along free).
    bqk = biasp.tile([128, 2, nh], F32, tag="bqk")
    nc.sync.dma_start(
        out=bqk[:, 0, :], in_=dram[f"bq_{s}"][:].rearrange("(j p) -> p j", p=128)
    )
    nc.sync.dma_start(
        out=bqk[:, 1, :], in_=dram[f"bk_{s}"][:].rearrange("(j p) -> p j", p=128)
    )
    bvb = biasp.tile([128, D], F32, tag="bvb")
    nc.sync.dma_start(out=bvb[:, :], in_=dram[f"bv_{s}"][:].unsqueeze(0).to_broadcast([128, D]))

    # ---- projections ----
    QT = qkvp.tile([128, nh, NTOK], BF16, tag="qt")  # [d, head, tok]
    KT = qkvp.tile([128, nh, NTOK], BF16, tag="kt")
    V = qkvp.tile([128, TT, D], BF16, tag="v")  # [tok, ttile, d]

    for which, (xname, wname, dst) in enumerate(
        (("xq", "wq", QT), ("xk", "wk", KT))
    ):
        xt = xtp.tile([128, CT, NTOK], BF16, tag="xt")  # transposed input
        xr = dram[f"{xname}_{s}"][:].rearrange("M (c p) -> M c p", p=128)
        for ct in range(CT):
            nc.sync.dma_start(out=xt[:, ct], in_=xr[:, ct], transpose=True)
        w = wp.tile([128, CT, D], BF16, tag=wname)
        nc.sync.dma_start(
            out=w[:, :, :],
            in_=dram[f"{wname}_{s}"][:].rearrange("(c p) d -> p c d", p=128),
        )
        # dst[d, tok] = sum_c w[c, d] * xt[c, tok]  (+ bias[d])
        for dt in range(nh):
            for qc in range(NQC):
                ps = ppv.tile([128, QCH], F32, tag="pv")
                for ct in range(CT):
                    nc.tensor.matmul(
                        ps[:, :],
                        w[:, ct, dt * 128 : (dt + 1) * 128],
                        xt[:, ct, qc * QCH : (qc + 1) * QCH],
                        start=(ct == 0),
                        stop=(ct == CT - 1),
                    )
                nc.vector.tensor_scalar_add(
                    dst[:, dt, qc * QCH : (qc + 1) * QCH],
                    ps[:, :],
                    bqk[:, which, dt : dt + 1],
                )

    # V natural layout: V[tok, d] = sum_c xt[c, tok] * w[c, d] (+ bv[d])
    xt = xtp.tile([128, CT, NTOK], BF16, tag="xt")
    xr = dram[f"xv_{s}"][:].rearrange("M (c p) -> M c p", p=128)
    for ct in range(CT):
        nc.sync.dma_start(out=xt[:, ct], in_=xr[:, ct], transpose=True)
    w = wp.tile([128, CT, D], BF16, tag="wv")
    nc.sync.dma_start(
        out=w[:, :, :], in_=dram[f"wv_{s}"][:].rearrange("(c p) d -> p c d", p=128)
    )
    for tt in range(TT):
        ps = ppv.tile([128, D], F32, tag="pv")
        for ct in range(CT):
            nc.tensor.matmul(
                ps[:, :],
                xt[:, ct, tt * 128 : (tt + 1) * 128],
                w[:, ct, :],
                start=(ct == 0),
                stop=(ct == CT - 1),
            )
        nc.vector.tensor_add(V[:, tt, :], ps[:, :], bvb[:, :])

    # ---- attention ----
    for h in range(nh):
        for qc in range(NQC):
            qsl = slice(qc * QCH, (qc + 1) * QCH)
            # E[k, q] = exp(scale * sum_d KT[d, k] QT[d, q]), k-tiled
            E = ep.tile([128, TT, QCH], BF16, tag="E")
            for g in range(TT // 2):
                st = pst.tile([128, 2 * QCH], F32, tag="st")
                for j in range(2):
                    kt = 2 * g + j
                    nc.tensor.matmul(
                        st[:, j * QCH : (j + 1) * QCH],
                        KT[:, h, kt * 128 : (kt + 1) * 128],
                        QT[:, h, qsl],
                        start=True,
                        stop=True,
                    )
                nc.scalar.activation(
                    E[:, 2 * g : 2 * g + 2, :],
                    st[:, :].rearrange("p (a b) -> p a b", b=QCH),
                    mybir.ActivationFunctionType.Exp,
                    scale=SCALE,
                )
            # denominator: bf16 tree-sum over the 16 k-tiles (free axis,
            # all-SBUF bf16 keeps the DVE 2x fast path) ...
            acc = accp.tile([128, 8, QCH], BF16, tag="acc")
            nc.vector.tensor_add(acc[:, 0:8, :], E[:, 0:8, :], E[:, 8:16, :])
            nc.vector.tensor_add(acc[:, 0:4, :], acc[:, 0:4, :], acc[:, 4:8, :])
            nc.vector.tensor_add(acc[:, 0:2, :], acc[:, 0:2, :], acc[:, 2:4, :])
            nc.vector.tensor_add(acc[:, 0:1, :], acc[:, 0:1, :], acc[:, 1:2, :])
            # ... then PE-transpose each 128-wide piece of the remaining row
            # so the partition-axis sum becomes a cheap free-axis DVE reduce,
            # yielding the denominator as a per-partition (per-q) column.
            # Division then folds into the final psum->sbuf copies as a
            # tensor_scalar multiply (a full [128,512] vector.reciprocal
            # would cost ~3.3us per chunk).
            dcol = recp.tile([128, NQC], F32, tag="dcol")
            for j in range(NQC):
                tpa = ptp.tile([128, 128], BF16, tag="tpa")
                nc.tensor.transpose(
                    tpa[:, :], acc[:, 0, j * 128 : (j + 1) * 128], identb[:, :]
                )
                nc.vector.reduce_sum(
                    out=dcol[:, j : j + 1], in_=tpa[:, :], axis=mybir.AxisListType.X
                )
            rec4 = recp.tile([128, NQC], F32, tag="rec4")
            nc.vector.reciprocal(rec4[:, :], dcol[:, :])

            # outT[d, q] = sum_k V[k, d] E[k, q]
            pv = ppv.tile([128, QCH], F32, tag="pv")
            for kt in range(TT):
                nc.tensor.matmul(
                    pv[:, :],
                    V[:, kt, h * 128 : (h + 1) * 128],
                    E[:, kt, :],
                    start=(kt == 0),
                    stop=(kt == TT - 1),
                )
            outT = recp.tile([128, QCH], F32, tag="outT")
            nc.scalar.copy(outT[:, :], pv[:, :])

            # transpose back to [q, d]; the softmax division is folded into
            # the psum->sbuf copy as a per-partition (per-q) scalar multiply
            ot = outp.tile([128, NQC, 128], F32, tag="ot")
            for j in range(NQC):
                tp = ptp.tile([128, 128], F32, tag="tp")
                nc.tensor.transpose(tp[:, :], outT[:, j * 128 : (j + 1) * 128], ident[:, :])
                nc.vector.tensor_scalar_mul(ot[:, j, :], tp[:, :], rec4[:, j : j + 1])
            nc.sync.dma_start(
                out=out_d[qc * QCH : (qc + 1) * QCH, h * 128 : (h + 1) * 128].rearrange(
                    "(j p) d -> p j d", p=128
                ),
                in_=ot[:, :, :],
            )


def _build_program():
    # Bacc (not plain Bass): its compile() pipeline legalizes multi-wait
    # instructions (walrus accepts at most 1 sync wait per instruction).
    nc = bacc.Bacc()
    dram = {}
    for s in ("a", "b"):
        D = 512 if s == "a" else 256
        for nm in ("xq", "xk", "xv"):
            dram[f"{nm}_{s}"] = nc.dram_tensor(
                f"{nm}_{s}", [NTOK, DIM], BF16, kind="ExternalInput"
            )
        for nm in ("wq", "wk", "wv"):
            dram[f"{nm}_{s}"] = nc.dram_tensor(
                f"{nm}_{s}", [DIM, D], BF16, kind="ExternalInput"
            )
        for nm in ("bq", "bk", "bv"):
            dram[f"{nm}_{s}"] = nc.dram_tensor(
                f"{nm}_{s}", [D], F32, kind="ExternalInput"
            )
        dram[f"out_{s}"] = nc.dram_tensor(
            f"out_{s}", [NTOK, D], F32, kind="ExternalOutput"
        )

    with tile.TileContext(nc) as tc:
        with (
            tc.tile_pool(name="singles", bufs=1) as singles,
            tc.tile_pool(name="xtp", bufs=2) as xtp,
            tc.tile_pool(name="qkvp", bufs=1) as qkvp,
            tc.tile_pool(name="wp", bufs=1) as wp,
            tc.tile_pool(name="ep", bufs=2) as ep,
            tc.tile_pool(name="accp", bufs=2) as accp,
            tc.tile_pool(name="recp", bufs=2) as recp,
            tc.tile_pool(name="outp", bufs=3) as outp,
            tc.tile_pool(name="biasp", bufs=1) as biasp,
            tc.tile_pool(name="pst", bufs=2, space="PSUM") as pst,
            tc.tile_pool(name="ppv", bufs=2, space="PSUM") as ppv,
            tc.tile_pool(name="ptp", bufs=1, space="PSUM") as ptp,
        ):
            ident = singles.tile([128, 128], F32, tag="ident")
            make_identity(nc, ident[:, :])
            identb = singles.tile([128, 128], BF16, tag="identb")
            make_identity(nc, identb[:, :])

            pools = (xtp, qkvp, wp, ep, accp, recp, outp, biasp, pst, ppv, ptp)
            _emit_slot(nc, pools, dram, "a", 4, ident, identb)
            _emit_slot(nc, pools, dram, "b", 2, ident, identb)

    # Run Bacc's compile pipeline (register allocation, sync-wait
    # legalization, nop fusion) — run_bass_via_pjrt does not call it.
    nc.finalize()
    return nc


_PROGRAM = None


def _get_program():
    global _PROGRAM
    if _PROGRAM is None:
        _PROGRAM = _build_program()
    return _PROGRAM


def kernel(query, key, value, Wq, bq, Wk, bk, Wv, bv):
    global LAST_RESULTS
    bf = ml_dtypes.bfloat16
    q = np.ascontiguousarray(np.asarray(query, np.float32).reshape(NBM, NTOK, DIM)).astype(bf)
    k = np.ascontiguousarray(np.asarray(key, np.float32).reshape(NBM, NTOK, DIM)).astype(bf)
    v = np.ascontiguousarray(np.asarray(value, np.float32).reshape(NBM, NTOK, DIM)).astype(bf)
    WqT = np.ascontiguousarray(np.asarray(Wq, np.float32).T).astype(bf)
    WkT = np.ascontiguousarray(np.asarray(Wk, np.float32).T).astype(bf)
    WvT = np.ascontiguousarray(np.asarray(Wv, np.float32).T).astype(bf)
    bq = np.asarray(bq, np.float32)
    bk = np.asarray(bk, np.float32)
    bv = np.asarray(bv, np.float32)

    in_maps = []
    for c in range(NCORES):
        bm_a = c
        bm_b = 8 + c // 2
        hs = (c % 2) * 256  # head-pair column offset for slot B
        in_maps.append(
            {
                "xq_a": q[bm_a], "xk_a": k[bm_a], "xv_a": v[bm_a],
                "xq_b": q[bm_b], "xk_b": k[bm_b], "xv_b": v[bm_b],
                "wq_a": WqT, "wk_a": WkT, "wv_a": WvT,
                "bq_a": bq, "bk_a": bk, "bv_a": bv,
                "wq_b": np.ascontiguousarray(WqT[:, hs : hs + 256]),
                "wk_b": np.ascontiguousarray(WkT[:, hs : hs + 256]),
                "wv_b": np.ascontiguousarray(WvT[:, hs : hs + 256]),
                "bq_b": np.ascontiguousarray(bq[hs : hs + 256]),
                "bk_b": np.ascontiguousarray(bk[hs : hs + 256]),
                "bv_b": np.ascontiguousarray(bv[hs : hs + 256]),
            }
        )

    nc = _get_program()
    res = run_bass_kernel_spmd(
        nc, in_maps, list(range(NCORES)), trace=TRACE, **TRACE_KWARGS
    )
    LAST_RESULTS = res

    out = np.empty((NBM, NTOK, DIM), np.float32)
    for c in range(NCORES):
        hs = (c % 2) * 256
        out[c] = res.results[c]["out_a"]
        out[8 + c // 2][:, hs : hs + 256] = res.results[c]["out_b"]
    return out.reshape(B, M, NTOK, DIM)
